# revision 13
# baseline (speedup 1.0000x reference)
"""Trainium2 Bass kernel for nn_GroupDenseFull.

Math: z[b, t*8+v] = sum_{s,w} x[b, s*8+w] * kernel_seq[s,w,v] * kernel_full[s,t]
  == x @ Wc  with  Wc[(s,w),(t,v)] = kernel_seq[s,w,v] * kernel_full[s,t]

Sharding: data-parallel over batch across 8 cores (16384 rows each).

Kernel design ("fused stationary"): per 512-row chunk
  1. DMA load x chunk (128p x 4 x 1024) natural layout (batch on partitions).
  2. PE transpose-in: 32x (128b x 128c) -> xT tiles (c on partitions).
  3. PE matmul accumulation with xT as the *stationary* operand and Wc as the
     moving operand: out[b, c_out] = sum_k xT_k.T @ Wc[k] -- output lands
     directly in natural (batch-on-partitions) layout; no transpose-out.
  4. Evict PSUM -> SBUF, DMA store.
"""

import os
from contextlib import ExitStack

import numpy as np

import concourse.bass as bass
import concourse.tile as tile
from concourse import bacc, mybir
from concourse.bass_utils import run_bass_kernel_spmd
from concourse.masks import make_identity

B, C, W, S = 131072, 1024, 8, 128
NCORES = 8
BSH = B // NCORES          # 16384 rows per core
CH = 512                   # chunk rows
NCH = BSH // CH            # 32 chunks
NJ = CH // 128             # 4 batch subtiles per chunk
NK = C // 128              # 8 channel tiles

F32 = mybir.dt.float32
F32R = mybir.dt.float32r
BF16 = mybir.dt.bfloat16

# knobs
MM_DT = F32R               # dtype for the big accumulating matmuls
TP_DT = F32R               # dtype for the PE transposes

TRACE = bool(int(os.environ.get("KERNEL_TRACE", "0")))
LAST_EXEC_NS = None
LAST_TRACE_DIR = None

_cache = {}


def _setup_trace_shim():
    """The agent image lacks antenv.axon_hooks; register the NTFF profile
    hook ourselves so run_bass_kernel_spmd(trace=True) works."""
    import sys
    import types

    import antenv
    from trn_agent_boot.trn_boot import _ntff_profile_via_ctypes

    if "antenv.axon_hooks" in sys.modules:
        return
    mod = types.ModuleType("antenv.axon_hooks")
    mod._hook = _ntff_profile_via_ctypes("/opt/axon/libaxon_pjrt.so")
    mod.get_axon_ntff_profile_hook = lambda: mod._hook
    mod.set_axon_ntff_profile_hook = lambda h: setattr(mod, "_hook", h)
    sys.modules["antenv.axon_hooks"] = mod
    antenv.axon_hooks = mod
    # no bucket in this container; keep artifacts local
    import concourse.bass_utils as bu

    bu.upload_artifacts = lambda tmpdir: tmpdir


def _build():
    nc = bacc.Bacc(
        "TRN2", target_bir_lowering=False, debug=False, num_devices=NCORES
    )
    x_ap = nc.dram_tensor("x", [BSH, C], F32R, kind="ExternalInput").ap()
    wc_ap = nc.dram_tensor("wc", [C, C], F32R, kind="ExternalInput").ap()
    id_ap = nc.dram_tensor("ident", [128, 128], F32R, kind="ExternalInput").ap()
    z_ap = nc.dram_tensor("z", [BSH, C], F32, kind="ExternalOutput").ap()

    with tile.TileContext(nc) as tc, ExitStack() as ctx:
        consts = ctx.enter_context(tc.tile_pool(name="consts", bufs=1))
        ident = consts.tile([128, 128], F32R)
        nc.sync.dma_start(ident, id_ap)
        wc_sb = consts.tile([128, NK, C], F32R)  # [p, k, c_out] 4MB
        nc.sync.dma_start(wc_sb, wc_ap.rearrange("(k p) c -> p k c", p=128))

        xpool = ctx.enter_context(tc.tile_pool(name="x", bufs=2))
        xtpool = ctx.enter_context(tc.tile_pool(name="xt", bufs=2))
        zpool = ctx.enter_context(tc.tile_pool(name="z", bufs=2))
        pst = ctx.enter_context(tc.tile_pool(name="pst", bufs=2, space="PSUM"))
        psz = ctx.enter_context(tc.tile_pool(name="psz", bufs=3, space="PSUM"))

        for c in range(NCH):
            # x split into halves for finer DMA->compute pipelining
            x_h = []
            for g in range(2):
                xg = xpool.tile([128, 2, C], F32R, tag=f"x{g}")
                nc.sync.dma_start(
                    xg,
                    x_ap[c * CH + g * 256:c * CH + (g + 1) * 256, :].rearrange(
                        "(j p) c -> p j c", p=128
                    ),
                )
                x_h.append(xg)

            # transpose-in: per-k tiles so matmuls start as soon as their
            # slice is evicted
            xts = []
            for k in range(NK):
                xtk = xtpool.tile([128, CH], F32R, tag=f"xt{k}")
                tpb = pst.tile([128, CH], F32R)
                for j in range(NJ):
                    nc.tensor.transpose(
                        tpb[:, j * 128:(j + 1) * 128],
                        x_h[j // 2][:, j % 2, k * 128:(k + 1) * 128],
                        ident,
                    )
                if k % 2 == 0:
                    nc.vector.tensor_copy(out=xtk, in_=tpb)
                else:
                    nc.scalar.copy(out=xtk, in_=tpb)
                xts.append(xtk)

            # fused matmul: z_nat[b, :] += xT_k.T @ Wc[k, :]
            z_h = []
            for g in range(2):
                zg = zpool.tile([128, 2, C], F32, tag=f"z{g}")
                z_h.append(zg)
            for j in range(NJ):
                zp = psz.tile([128, C], F32)  # 2 PSUM banks
                for h in range(2):
                    for k in range(NK):
                        nc.tensor.matmul(
                            zp[:, h * 512:(h + 1) * 512],
                            xts[k][:, j * 128:(j + 1) * 128],
                            wc_sb[:, k, h * 512:(h + 1) * 512],
                            start=(k == 0),
                            stop=(k == NK - 1),
                        )
                if j % 2 == 0:
                    nc.vector.tensor_copy(out=z_h[j // 2][:, j % 2, :], in_=zp)
                else:
                    nc.scalar.copy(out=z_h[j // 2][:, j % 2, :], in_=zp)
            for g in range(2):
                nc.sync.dma_start(
                    z_ap[c * CH + g * 256:c * CH + (g + 1) * 256, :].rearrange(
                        "(j p) c -> p j c", p=128
                    ),
                    z_h[g],
                )

    nc.compile()
    return nc


def kernel(x, kernel_seq, kernel_full):
    global LAST_EXEC_NS
    x = np.ascontiguousarray(np.asarray(x, dtype=np.float32))
    ks = np.asarray(kernel_seq, dtype=np.float32)
    kf = np.asarray(kernel_full, dtype=np.float32)
    # Wc[(s,w),(t,v)] = ks[s,w,v] * kf[s,t]
    wc = np.einsum("swv,st->swtv", ks, kf).reshape(C, C)
    wc = np.ascontiguousarray(wc)

    if "nc" not in _cache:
        _cache["nc"] = _build()
    nc = _cache["nc"]

    xs = x.reshape(NCORES, BSH, C)
    ident = np.ascontiguousarray(np.eye(128, dtype=np.float32))
    in_maps = [{"x": xs[i], "wc": wc, "ident": ident} for i in range(NCORES)]
    kw = {}
    if TRACE:
        _setup_trace_shim()
        global LAST_TRACE_DIR
        import tempfile

        LAST_TRACE_DIR = tempfile.mkdtemp(prefix="ktrace_")
        kw = {"tmpdir": LAST_TRACE_DIR}
    res = run_bass_kernel_spmd(nc, in_maps, list(range(NCORES)), trace=TRACE, **kw)
    if res.exec_time_ns is not None:
        LAST_EXEC_NS = res.exec_time_ns
    z = np.concatenate([r["z"] for r in res.results], axis=0)
    return np.ascontiguousarray(z.astype(np.float32))


# revision 14
# speedup vs baseline: 1.0055x; 1.0055x over previous
"""Trainium2 Bass kernel for nn_GroupDenseFull.

Math: z[b, t*8+v] = sum_{s,w} x[b, s*8+w] * kernel_seq[s,w,v] * kernel_full[s,t]
  == x @ Wc  with  Wc[(s,w),(t,v)] = kernel_seq[s,w,v] * kernel_full[s,t]

Sharding: data-parallel over batch across 8 cores (16384 rows each).

Kernel design ("fused stationary"): per 512-row chunk
  1. DMA load x chunk (128p x 4 x 1024) natural layout (batch on partitions).
  2. PE transpose-in: 32x (128b x 128c) -> xT tiles (c on partitions).
  3. PE matmul accumulation with xT as the *stationary* operand and Wc as the
     moving operand: out[b, c_out] = sum_k xT_k.T @ Wc[k] -- output lands
     directly in natural (batch-on-partitions) layout; no transpose-out.
  4. Evict PSUM -> SBUF, DMA store.
"""

import os
from contextlib import ExitStack

import numpy as np

import concourse.bass as bass
import concourse.tile as tile
from concourse import bacc, mybir
from concourse.bass_utils import run_bass_kernel_spmd
from concourse.masks import make_identity

B, C, W, S = 131072, 1024, 8, 128
NCORES = 8
BSH = B // NCORES          # 16384 rows per core
CH = 512                   # chunk rows
NCH = BSH // CH            # 32 chunks
NJ = CH // 128             # 4 batch subtiles per chunk
NK = C // 128              # 8 channel tiles

F32 = mybir.dt.float32
F32R = mybir.dt.float32r
BF16 = mybir.dt.bfloat16

# knobs
MM_DT = F32R               # dtype for the big accumulating matmuls
TP_DT = F32R               # dtype for the PE transposes

TRACE = bool(int(os.environ.get("KERNEL_TRACE", "0")))
LAST_EXEC_NS = None
LAST_TRACE_DIR = None

_cache = {}


def _setup_trace_shim():
    """The agent image lacks antenv.axon_hooks; register the NTFF profile
    hook ourselves so run_bass_kernel_spmd(trace=True) works."""
    import sys
    import types

    import antenv
    from trn_agent_boot.trn_boot import _ntff_profile_via_ctypes

    if "antenv.axon_hooks" in sys.modules:
        return
    mod = types.ModuleType("antenv.axon_hooks")
    mod._hook = _ntff_profile_via_ctypes("/opt/axon/libaxon_pjrt.so")
    mod.get_axon_ntff_profile_hook = lambda: mod._hook
    mod.set_axon_ntff_profile_hook = lambda h: setattr(mod, "_hook", h)
    sys.modules["antenv.axon_hooks"] = mod
    antenv.axon_hooks = mod
    # no bucket in this container; keep artifacts local
    import concourse.bass_utils as bu

    bu.upload_artifacts = lambda tmpdir: tmpdir


def _build():
    nc = bacc.Bacc(
        "TRN2", target_bir_lowering=False, debug=False, num_devices=NCORES
    )
    x_ap = nc.dram_tensor("x", [BSH, C], F32R, kind="ExternalInput").ap()
    wc_ap = nc.dram_tensor("wc", [C, C], F32R, kind="ExternalInput").ap()
    id_ap = nc.dram_tensor("ident", [128, 128], F32R, kind="ExternalInput").ap()
    z_ap = nc.dram_tensor("z", [BSH, C], F32, kind="ExternalOutput").ap()

    with tile.TileContext(nc) as tc, ExitStack() as ctx:
        consts = ctx.enter_context(tc.tile_pool(name="consts", bufs=1))
        ident = consts.tile([128, 128], F32R)
        nc.sync.dma_start(ident, id_ap)
        wc_sb = consts.tile([128, NK, C], F32R)  # [p, k, c_out] 4MB
        nc.sync.dma_start(wc_sb, wc_ap.rearrange("(k p) c -> p k c", p=128))

        xpool = ctx.enter_context(tc.tile_pool(name="x", bufs=2))
        xtpool = ctx.enter_context(tc.tile_pool(name="xt", bufs=2))
        zpool = ctx.enter_context(tc.tile_pool(name="z", bufs=2))
        pst = ctx.enter_context(tc.tile_pool(name="pst", bufs=2, space="PSUM"))
        psz = ctx.enter_context(tc.tile_pool(name="psz", bufs=3, space="PSUM"))

        for c in range(NCH):
            # x split into halves for finer DMA->compute pipelining
            x_h = []
            for g in range(2):
                xg = xpool.tile([128, 2, C], F32R, tag=f"x{g}")
                nc.sync.dma_start(
                    xg,
                    x_ap[c * CH + g * 256:c * CH + (g + 1) * 256, :].rearrange(
                        "(j p) c -> p j c", p=128
                    ),
                )
                x_h.append(xg)

            # transpose-in: per-k tiles so matmuls start as soon as their
            # slice is evicted
            xts = []
            for k in range(NK):
                xtk = xtpool.tile([128, CH], F32R, tag=f"xt{k}")
                tpb = pst.tile([128, CH], F32R)
                for j in range(NJ):
                    nc.tensor.transpose(
                        tpb[:, j * 128:(j + 1) * 128],
                        x_h[j // 2][:, j % 2, k * 128:(k + 1) * 128],
                        ident,
                    )
                if k % 2 == 0:
                    nc.vector.tensor_copy(out=xtk, in_=tpb)
                else:
                    nc.scalar.copy(out=xtk, in_=tpb)
                xts.append(xtk)

            # fused matmul: z_nat[b, :] += xT_k.T @ Wc[k, :]
            z_h = []
            for g in range(2):
                zg = zpool.tile([128, 2, C], F32, tag=f"z{g}")
                z_h.append(zg)
            for j in range(NJ):
                zp = psz.tile([128, C], F32)  # 2 PSUM banks
                for k in range(NK):
                    lhsT = xts[k][:, j * 128:(j + 1) * 128]
                    for h in range(2):
                        nc.tensor.matmul(
                            zp[:, h * 512:(h + 1) * 512],
                            lhsT,
                            wc_sb[:, k, h * 512:(h + 1) * 512],
                            start=(k == 0),
                            stop=(k == NK - 1),
                        )
                if j % 2 == 0:
                    nc.vector.tensor_copy(out=z_h[j // 2][:, j % 2, :], in_=zp)
                else:
                    nc.scalar.copy(out=z_h[j // 2][:, j % 2, :], in_=zp)
            for g in range(2):
                nc.sync.dma_start(
                    z_ap[c * CH + g * 256:c * CH + (g + 1) * 256, :].rearrange(
                        "(j p) c -> p j c", p=128
                    ),
                    z_h[g],
                )

    nc.compile()
    return nc


def kernel(x, kernel_seq, kernel_full):
    global LAST_EXEC_NS
    x = np.ascontiguousarray(np.asarray(x, dtype=np.float32))
    ks = np.asarray(kernel_seq, dtype=np.float32)
    kf = np.asarray(kernel_full, dtype=np.float32)
    # Wc[(s,w),(t,v)] = ks[s,w,v] * kf[s,t]
    wc = np.einsum("swv,st->swtv", ks, kf).reshape(C, C)
    wc = np.ascontiguousarray(wc)

    if "nc" not in _cache:
        _cache["nc"] = _build()
    nc = _cache["nc"]

    xs = x.reshape(NCORES, BSH, C)
    ident = np.ascontiguousarray(np.eye(128, dtype=np.float32))
    in_maps = [{"x": xs[i], "wc": wc, "ident": ident} for i in range(NCORES)]
    kw = {}
    if TRACE:
        _setup_trace_shim()
        global LAST_TRACE_DIR
        import tempfile

        LAST_TRACE_DIR = tempfile.mkdtemp(prefix="ktrace_")
        kw = {"tmpdir": LAST_TRACE_DIR}
    res = run_bass_kernel_spmd(nc, in_maps, list(range(NCORES)), trace=TRACE, **kw)
    if res.exec_time_ns is not None:
        LAST_EXEC_NS = res.exec_time_ns
    z = np.concatenate([r["z"] for r in res.results], axis=0)
    return np.ascontiguousarray(z.astype(np.float32))


# revision 15
# speedup vs baseline: 1.0068x; 1.0013x over previous
"""Trainium2 Bass kernel for nn_GroupDenseFull.

Math: z[b, t*8+v] = sum_{s,w} x[b, s*8+w] * kernel_seq[s,w,v] * kernel_full[s,t]
  == x @ Wc  with  Wc[(s,w),(t,v)] = kernel_seq[s,w,v] * kernel_full[s,t]

Sharding: data-parallel over batch across 8 cores (16384 rows each).

Kernel design ("fused stationary"): per 512-row chunk
  1. DMA load x chunk (128p x 4 x 1024) natural layout (batch on partitions).
  2. PE transpose-in: 32x (128b x 128c) -> xT tiles (c on partitions).
  3. PE matmul accumulation with xT as the *stationary* operand and Wc as the
     moving operand: out[b, c_out] = sum_k xT_k.T @ Wc[k] -- output lands
     directly in natural (batch-on-partitions) layout; no transpose-out.
  4. Evict PSUM -> SBUF, DMA store.
"""

import os
from contextlib import ExitStack

import numpy as np

import concourse.bass as bass
import concourse.tile as tile
from concourse import bacc, mybir
from concourse.bass_utils import run_bass_kernel_spmd
from concourse.masks import make_identity

B, C, W, S = 131072, 1024, 8, 128
NCORES = 8
BSH = B // NCORES          # 16384 rows per core
CH = 512                   # chunk rows
NCH = BSH // CH            # 32 chunks
NJ = CH // 128             # 4 batch subtiles per chunk
NK = C // 128              # 8 channel tiles

F32 = mybir.dt.float32
F32R = mybir.dt.float32r
BF16 = mybir.dt.bfloat16

# knobs
MM_DT = F32R               # dtype for the big accumulating matmuls
TP_DT = F32R               # dtype for the PE transposes

TRACE = bool(int(os.environ.get("KERNEL_TRACE", "0")))
LAST_EXEC_NS = None
LAST_TRACE_DIR = None

_cache = {}


def _setup_trace_shim():
    """The agent image lacks antenv.axon_hooks; register the NTFF profile
    hook ourselves so run_bass_kernel_spmd(trace=True) works."""
    import sys
    import types

    import antenv
    from trn_agent_boot.trn_boot import _ntff_profile_via_ctypes

    if "antenv.axon_hooks" in sys.modules:
        return
    mod = types.ModuleType("antenv.axon_hooks")
    mod._hook = _ntff_profile_via_ctypes("/opt/axon/libaxon_pjrt.so")
    mod.get_axon_ntff_profile_hook = lambda: mod._hook
    mod.set_axon_ntff_profile_hook = lambda h: setattr(mod, "_hook", h)
    sys.modules["antenv.axon_hooks"] = mod
    antenv.axon_hooks = mod
    # no bucket in this container; keep artifacts local
    import concourse.bass_utils as bu

    bu.upload_artifacts = lambda tmpdir: tmpdir


def _build():
    nc = bacc.Bacc(
        "TRN2", target_bir_lowering=False, debug=False, num_devices=NCORES
    )
    x_ap = nc.dram_tensor("x", [BSH, C], F32R, kind="ExternalInput").ap()
    wc_ap = nc.dram_tensor("wc", [C, C], F32R, kind="ExternalInput").ap()
    id_ap = nc.dram_tensor("ident", [128, 128], F32R, kind="ExternalInput").ap()
    z_ap = nc.dram_tensor("z", [BSH, C], F32, kind="ExternalOutput").ap()

    with tile.TileContext(nc) as tc, ExitStack() as ctx:
        consts = ctx.enter_context(tc.tile_pool(name="consts", bufs=1))
        ident = consts.tile([128, 128], F32R)
        nc.sync.dma_start(ident, id_ap)
        wc_sb = consts.tile([128, NK, C], F32R)  # [p, k, c_out] 4MB
        nc.sync.dma_start(wc_sb, wc_ap.rearrange("(k p) c -> p k c", p=128))

        xpool = ctx.enter_context(tc.tile_pool(name="x", bufs=3))
        xtpool = ctx.enter_context(tc.tile_pool(name="xt", bufs=2))
        zpool = ctx.enter_context(tc.tile_pool(name="z", bufs=3))
        pst = ctx.enter_context(tc.tile_pool(name="pst", bufs=2, space="PSUM"))
        psz = ctx.enter_context(tc.tile_pool(name="psz", bufs=3, space="PSUM"))

        for c in range(NCH):
            # x split into halves for finer DMA->compute pipelining
            x_h = []
            for g in range(2):
                xg = xpool.tile([128, 2, C], F32R, tag=f"x{g}")
                nc.sync.dma_start(
                    xg,
                    x_ap[c * CH + g * 256:c * CH + (g + 1) * 256, :].rearrange(
                        "(j p) c -> p j c", p=128
                    ),
                )
                x_h.append(xg)

            # transpose-in: per-k tiles so matmuls start as soon as their
            # slice is evicted
            xts = []
            for k in range(NK):
                xtk = xtpool.tile([128, CH], F32R, tag=f"xt{k}")
                tpb = pst.tile([128, CH], F32R)
                for j in range(NJ):
                    nc.tensor.transpose(
                        tpb[:, j * 128:(j + 1) * 128],
                        x_h[j // 2][:, j % 2, k * 128:(k + 1) * 128],
                        ident,
                    )
                if k % 2 == 0:
                    nc.vector.tensor_copy(out=xtk, in_=tpb)
                else:
                    nc.scalar.copy(out=xtk, in_=tpb)
                xts.append(xtk)

            # fused matmul: z_nat[b, :] += xT_k.T @ Wc[k, :]
            z_h = []
            for g in range(2):
                zg = zpool.tile([128, 2, C], F32, tag=f"z{g}")
                z_h.append(zg)
            for j in range(NJ):
                zp = psz.tile([128, C], F32)  # 2 PSUM banks
                for k in range(NK):
                    lhsT = xts[k][:, j * 128:(j + 1) * 128]
                    for h in range(2):
                        nc.tensor.matmul(
                            zp[:, h * 512:(h + 1) * 512],
                            lhsT,
                            wc_sb[:, k, h * 512:(h + 1) * 512],
                            start=(k == 0),
                            stop=(k == NK - 1),
                        )
                if j % 2 == 0:
                    nc.vector.tensor_copy(out=z_h[j // 2][:, j % 2, :], in_=zp)
                else:
                    nc.scalar.copy(out=z_h[j // 2][:, j % 2, :], in_=zp)
            for g in range(2):
                nc.sync.dma_start(
                    z_ap[c * CH + g * 256:c * CH + (g + 1) * 256, :].rearrange(
                        "(j p) c -> p j c", p=128
                    ),
                    z_h[g],
                )

    nc.compile()
    return nc


def kernel(x, kernel_seq, kernel_full):
    global LAST_EXEC_NS
    x = np.ascontiguousarray(np.asarray(x, dtype=np.float32))
    ks = np.asarray(kernel_seq, dtype=np.float32)
    kf = np.asarray(kernel_full, dtype=np.float32)
    # Wc[(s,w),(t,v)] = ks[s,w,v] * kf[s,t]
    wc = np.einsum("swv,st->swtv", ks, kf).reshape(C, C)
    wc = np.ascontiguousarray(wc)

    if "nc" not in _cache:
        _cache["nc"] = _build()
    nc = _cache["nc"]

    xs = x.reshape(NCORES, BSH, C)
    ident = np.ascontiguousarray(np.eye(128, dtype=np.float32))
    in_maps = [{"x": xs[i], "wc": wc, "ident": ident} for i in range(NCORES)]
    kw = {}
    if TRACE:
        _setup_trace_shim()
        global LAST_TRACE_DIR
        import tempfile

        LAST_TRACE_DIR = tempfile.mkdtemp(prefix="ktrace_")
        kw = {"tmpdir": LAST_TRACE_DIR}
    res = run_bass_kernel_spmd(nc, in_maps, list(range(NCORES)), trace=TRACE, **kw)
    if res.exec_time_ns is not None:
        LAST_EXEC_NS = res.exec_time_ns
    z = np.concatenate([r["z"] for r in res.results], axis=0)
    return np.ascontiguousarray(z.astype(np.float32))
